# revision 15
# baseline (speedup 1.0000x reference)
"""nn_ComplexNetAttention on 8 trn2 NeuronCores — v2.

Sharding: heads column-parallel for QKV+attention (2 heads/core), per-head
fp16 AllToAll to redistribute attention output to token-sharded layout
(overlapping head-1 compute), token-parallel o-projection (256 tokens/core).

vs v1:
- int8 fake-quant of hidden computed on HOST; device receives dequantized
  fp16 activations and their r+i sum — removes the device quant phase.
- 3-matmul Karatsuba for all complex projections (QKV and O).
- softmax exp tiles in bf16 (fp16 overflows: max score*scale ~ 14).
- fp16 A2A payload split per head, overlapped with attention compute.
- o-proj weights preloaded / double-buffered ahead of use.
"""
import numpy as np
import ml_dtypes

import concourse.bass as bass
import concourse.bacc as bacc
import concourse.tile as tile
import concourse.mybir as mybir
from concourse.bass_utils import run_bass_kernel_spmd

f32 = mybir.dt.float32
f16 = mybir.dt.float16
bf16 = mybir.dt.bfloat16

T, H, NH, D = 2048, 2048, 16, 128
NC = 8
HPC = NH // NC          # heads per core = 2
DS = HPC * D            # d_out slice per core = 256
TS = T // NC            # tokens per core for o-proj = 256
CH = 256                # token chunk in projection phase
NCH = T // CH           # 8
R2 = 2 * D + 2          # per-src rows in per-head A2A payload = 258
MAGIC = float(2**23 + 2**22)  # fp32 round-to-nearest-even integer trick
A_OP = mybir.AluOpType
HT = H // 128           # 16


def build_nc():
    nc = bacc.Bacc("TRN2", target_bir_lowering=False, debug=False, num_devices=NC)
    A = {}
    def inp(name, shape, dt=f16):
        A[name] = nc.dram_tensor(name, shape, dt, kind="ExternalInput").ap()
    inp("xr", [H, T]); inp("xi", [H, T]); inp("xs", [H, T])
    for t_ in ("q", "k", "v"):
        for m in (1, 2, 3):
            inp(f"w{t_}{m}", [H, DS])
    for m in (1, 2, 3):
        inp(f"wo{m}", [H, H])
    inp("cosT", [D, T]); inp("sinT", [D, T])
    inp("masks", [128, 4 * 512])
    inp("ident", [128, 128])
    A["yr_part"] = nc.dram_tensor("yr_part", [TS, H], f32, kind="ExternalOutput").ap()
    A["yi_part"] = nc.dram_tensor("yi_part", [TS, H], f32, kind="ExternalOutput").ap()
    return nc, A


def _chunked(ap):
    """DRAM [H, w] -> [128, HT, w] view (partition, h-chunk, col)."""
    return ap.rearrange("(a b) c -> b a c", b=128)


def emit(nc, A, tc, ctx):
    const = ctx.enter_context(tc.tile_pool(name="const", bufs=1))
    ps = ctx.enter_context(tc.tile_pool(name="ps", bufs=1, space="PSUM"))
    dram = ctx.enter_context(tc.tile_pool(name="dram", bufs=1, space="DRAM"))

    ident = const.tile([128, 128], f16, name="ident_t")
    nc.sync.dma_start(ident[:], A["ident"][:])
    masks = const.tile([128, 4 * 512], f16, name="masks_t")
    nc.sync.dma_start(masks[:], A["masks"][:])

    # rotating psum allocator over 8 bank tags (max slot shape 128x512 f32)
    _pn = [0]
    def psum(shape=(128, 512)):
        t = ps.tile(list(shape), f32, name=f"pt{_pn[0]}", tag=f"p{_pn[0] % 8}")
        _pn[0] += 1
        return t
    def psum_at(k, shape=(128, 512), dt=f32):
        t = ps.tile(list(shape), dt, name=f"pta{_pn[0]}_{k}", tag=f"p{k % 8}")
        return t

    cont = [dram.tile([NC * R2, TS], f16, name=f"cont{h}") for h in range(HPC)]
    ag = [dram.tile([NC * R2, TS], f16, name=f"ag{h}") for h in range(HPC)]
    bounce = dram.tile([4, TS], f32, name="bounce")

    wo_pool_box = []
    def load_wo_jb(jb):
        ws = []
        for m in (1, 2, 3):
            w = wo_pool_box[0].tile([128, HT, 512], f16, name=f"wo{m}_{jb}", tag=f"wo{m}")
            nc.sync.dma_start(w[:], _chunked(A[f"wo{m}"][:, jb * 512:(jb + 1) * 512]))
            ws.append(w)
        return ws

    with tc.tile_pool(name="qk", bufs=1) as qk_pool:
        qrot = {}
        for tn in ("q", "k"):
            for hd in range(HPC):
                for cp in ("r", "i"):
                    qrot[(tn, hd, cp)] = qk_pool.tile([128, T], f16, name=f"{tn}rot{hd}{cp}")
        vjoin = {}
        for bk in range(T // 128):
            for hd in range(HPC):
                vjoin[(hd, bk)] = qk_pool.tile([128, 257], bf16, name=f"vj{hd}_{bk}")
                nc.vector.memset(vjoin[(hd, bk)][:, 256:257], 1.0)

        # ======== phase 1: QKV projections (Karatsuba), 256-token chunks ======
        with tc.tile_pool(name="wqk", bufs=1) as wqk_pool, \
             tc.tile_pool(name="acts", bufs=2) as acts_pool, \
             tc.tile_pool(name="cmb", bufs=2) as cmb:
            wqk = {}
            for tn in ("q", "k", "v"):
                for m in (1, 2, 3):
                    w = wqk_pool.tile([128, HT, DS], f16, name=f"w{tn}{m}_t")
                    nc.sync.dma_start(w[:], _chunked(A[f"w{tn}{m}"][:]))
                    wqk[(tn, m)] = w
            cosT = wqk_pool.tile([D, T], f16, name="cosT_t")
            nc.sync.dma_start(cosT[:], A["cosT"][:])
            sinT = wqk_pool.tile([D, T], f16, name="sinT_t")
            nc.sync.dma_start(sinT[:], A["sinT"][:])

            for ch in range(NCH):
                t0 = ch * CH
                csl = slice(t0, t0 + CH)
                acts = {}
                for cp in ("r", "i", "s"):
                    a = acts_pool.tile([128, HT, CH], f16, name=f"acts{cp}{ch}",
                                       tag=f"acts{cp}")
                    nc.sync.dma_start(a[:], _chunked(A[f"x{cp}"][:, csl]))
                    acts[cp] = a

                # Q/K: out[d_out 128, tok CH]; M1=Wi*xs, M2=(Wr-Wi)*xr, M3=(Wr+Wi)*xi
                for tn in ("q", "k"):
                    for dt_ in range(HPC):
                        dsl = slice(dt_ * 128, dt_ * 128 + 128)
                        p1, p2, p3 = psum((128, CH)), psum((128, CH)), psum((128, CH))
                        for h in range(HT):
                            st = (h == 0); sp = (h == HT - 1)
                            nc.tensor.matmul(p1[:], wqk[(tn, 1)][:, h, dsl], acts["s"][:, h, :],
                                             start=st, stop=sp)
                        for h in range(HT):
                            st = (h == 0); sp = (h == HT - 1)
                            nc.tensor.matmul(p2[:], wqk[(tn, 2)][:, h, dsl], acts["r"][:, h, :],
                                             start=st, stop=sp)
                        for h in range(HT):
                            st = (h == 0); sp = (h == HT - 1)
                            nc.tensor.matmul(p3[:], wqk[(tn, 3)][:, h, dsl], acts["i"][:, h, :],
                                             start=st, stop=sp)
                        c1 = cmb.tile([128, CH], f32, name=f"c1{tn}{dt_}{ch}", tag="c1")
                        nc.scalar.activation(c1[:], p1[:], mybir.ActivationFunctionType.Copy)
                        y_r = cmb.tile([128, CH], f32, name=f"yr{tn}{dt_}{ch}", tag="yr")
                        y_i = cmb.tile([128, CH], f32, name=f"yi{tn}{dt_}{ch}", tag="yi")
                        nc.vector.tensor_tensor(y_r[:], c1[:], p2[:], A_OP.add)
                        nc.vector.tensor_tensor(y_i[:], c1[:], p3[:], A_OP.subtract)
                        t1 = cmb.tile([128, CH], f32, name=f"t1{tn}{dt_}{ch}", tag="t1")
                        t2 = cmb.tile([128, CH], f32, name=f"t2{tn}{dt_}{ch}", tag="t2")
                        nc.vector.tensor_tensor(t1[:], y_r[:], cosT[:, csl], A_OP.mult)
                        nc.vector.tensor_tensor(t2[:], y_i[:], sinT[:, csl], A_OP.mult)
                        nc.vector.tensor_tensor(qrot[(tn, dt_, "r")][:, csl], t1[:], t2[:],
                                                A_OP.subtract)
                        nc.vector.tensor_tensor(t1[:], y_i[:], cosT[:, csl], A_OP.mult)
                        nc.vector.tensor_tensor(t2[:], y_r[:], sinT[:, csl], A_OP.mult)
                        nc.vector.tensor_tensor(qrot[(tn, dt_, "i")][:, csl], t1[:], t2[:],
                                                A_OP.add)

                # V: out[tok 128, d_out DS]; M1=xs*Vi, M2=xr*(Vr-Vi), M3=xi*(Vr+Vi)
                for tt in range(CH // 128):
                    bk = ch * (CH // 128) + tt
                    tsl = slice(tt * 128, tt * 128 + 128)
                    p1, p2, p3 = psum((128, DS)), psum((128, DS)), psum((128, DS))
                    for h in range(HT):
                        st = (h == 0); sp = (h == HT - 1)
                        nc.tensor.matmul(p1[:], acts["s"][:, h, tsl], wqk[("v", 1)][:, h, :],
                                         start=st, stop=sp)
                    for h in range(HT):
                        st = (h == 0); sp = (h == HT - 1)
                        nc.tensor.matmul(p2[:], acts["r"][:, h, tsl], wqk[("v", 2)][:, h, :],
                                         start=st, stop=sp)
                    for h in range(HT):
                        st = (h == 0); sp = (h == HT - 1)
                        nc.tensor.matmul(p3[:], acts["i"][:, h, tsl], wqk[("v", 3)][:, h, :],
                                         start=st, stop=sp)
                    cv = cmb.tile([128, DS], f32, name=f"cv{ch}{tt}", tag="cv")
                    nc.scalar.activation(cv[:], p1[:], mybir.ActivationFunctionType.Copy)
                    for hd in range(HPC):
                        hsl = slice(hd * 128, hd * 128 + 128)
                        nc.vector.tensor_tensor(vjoin[(hd, bk)][:, 0:128],
                                                cv[:, hsl], p2[:, hsl], A_OP.add)
                        nc.vector.tensor_tensor(vjoin[(hd, bk)][:, 128:256],
                                                cv[:, hsl], p3[:, hsl], A_OP.subtract)

        with tc.tile_pool(name="wo", bufs=2) as wo_pool:
            wo_pool_box.append(wo_pool)
            # preload first o-proj weight block (runs during attention)
            wo_pre = {0: load_wo_jb(0)}

            # ======== phase 2: attention per head + per-head AllToAll ========
            SC = float(1.0 / np.sqrt(2 * D))
            with tc.tile_pool(name="attn", bufs=1) as at, \
                 tc.tile_pool(name="epool", bufs=2) as ep, \
                 tc.tile_pool(name="tp", bufs=2) as tp:
                for hd in range(HPC):
                    out_nat = {}
                    for g in range(4):
                        etiles = {}
                        for bk in range(4 * g + 4):
                            pS = psum_at(bk % 4)
                            qsl = slice(g * 512, g * 512 + 512)
                            nc.tensor.matmul(pS[:],
                                             qrot[("k", hd, "r")][:, bk * 128:bk * 128 + 128],
                                             qrot[("q", hd, "r")][:, qsl],
                                             start=True, stop=False)
                            nc.tensor.matmul(pS[:],
                                             qrot[("k", hd, "i")][:, bk * 128:bk * 128 + 128],
                                             qrot[("q", hd, "i")][:, qsl],
                                             start=False, stop=True)
                            if bk >= 4 * g:
                                mc = (bk - 4 * g) * 512
                                nc.vector.tensor_tensor(pS[:], pS[:], masks[:, mc:mc + 512],
                                                        A_OP.add)
                            e = ep.tile([128, 512], bf16, name=f"e{hd}{g}_{bk}", tag=f"e{bk}")
                            nc.scalar.activation(e[:], pS[:], mybir.ActivationFunctionType.Exp,
                                                 scale=SC)
                            etiles[bk] = e
                        for bq in range(4 * g, 4 * g + 4):
                            pO = psum_at(4 + bq % 4, (128, 257))
                            col = (bq - 4 * g) * 128
                            for bk in range(bq + 1):
                                nc.tensor.matmul(pO[:], etiles[bk][:, col:col + 128],
                                                 vjoin[(hd, bk)][:],
                                                 start=(bk == 0), stop=(bk == bq))
                            rec = at.tile([128, 1], f32, name=f"rec{hd}{bq}", tag="rec")
                            nc.vector.reciprocal(rec[:], pO[:, 256:257])
                            for ci, cp in enumerate(("r", "i")):
                                o = at.tile([128, 128], f16, name=f"on{hd}{cp}{bq}",
                                            tag=f"on{cp}{bq}")
                                nc.vector.tensor_scalar(o[:], pO[:, ci * 128:ci * 128 + 128],
                                                        rec[:], None, A_OP.mult)
                                out_nat[(cp, bq)] = o
                                mx = at.tile([128, 1], f16, name=f"mx{hd}{cp}{bq}", tag="mxt")
                                nc.vector.tensor_reduce(mx[:], o[:], mybir.AxisListType.X,
                                                        A_OP.max, apply_absolute_value=True)
                                s_, c0 = bq // 2, (bq % 2) * 128
                                dst = cont[hd][s_ * R2 + 2 * D + ci: s_ * R2 + 2 * D + ci + 1,
                                               c0:c0 + 128]
                                nc.sync.dma_start(dst.rearrange("a b -> b a"), mx[:])

                    # transpose to [d, tok] and scatter into cont[hd]
                    for ci, cp in enumerate(("r", "i")):
                        oT = tp.tile([128, T], f16, name=f"oT{hd}{cp}", tag="oT")
                        for bq in range(T // 128):
                            pT = psum_at(bq % 2, (128, 128), f16)
                            nc.tensor.transpose(pT[:], out_nat[(cp, bq)][:], ident[:])
                            nc.vector.tensor_copy(oT[:, bq * 128:bq * 128 + 128], pT[:])
                        r0 = ci * 128
                        for s_ in range(NC):
                            nc.sync.dma_start(cont[hd][s_ * R2 + r0: s_ * R2 + r0 + 128, :],
                                              oT[:, s_ * TS:(s_ + 1) * TS])
                    nc.gpsimd.collective_compute(
                        "AllToAll", A_OP.bypass, replica_groups=[list(range(NC))],
                        ins=[cont[hd][:].opt()], outs=[ag[hd][:].opt()])

            # ======== phase 3: o-projection on my 256-token slice ========
            wo_pre[1] = load_wo_jb(1)
            with tc.tile_pool(name="op", bufs=1) as op, \
                 tc.tile_pool(name="od", bufs=2) as od:
                # global per-token absmax over 16 (core, head) sources
                gmax = {}
                for ci, cp in enumerate(("r", "i")):
                    g = op.tile([1, TS], f32, name=f"gmax{cp}")
                    first = True
                    for hd in range(HPC):
                        agrows = ag[hd][:].rearrange("(s r) c -> r s c", r=R2)
                        mrows = op.tile([1, NC, TS], f16, name=f"mrows{cp}{hd}", tag="mrows")
                        nc.sync.dma_start(mrows[:], agrows[2 * D + ci: 2 * D + ci + 1, :, :])
                        for s_ in range(NC):
                            if first:
                                nc.vector.tensor_tensor(g[:], mrows[:, s_, :], mrows[:, s_, :],
                                                        A_OP.max)
                                first = False
                            else:
                                nc.vector.tensor_tensor(g[:], g[:], mrows[:, s_, :], A_OP.max)
                    nc.vector.tensor_scalar(g[:], g[:], 1e-5, None, A_OP.max)
                    gmax[cp] = g
                srep_s = {}; srep_inv = {}
                for ci, cp in enumerate(("r", "i")):
                    rg = op.tile([1, TS], f32, name=f"rg{cp}")
                    nc.vector.reciprocal(rg[:], gmax[cp][:])
                    nc.vector.tensor_scalar(rg[:], rg[:], 127.0, None, A_OP.mult)
                    nc.sync.dma_start(bounce[ci:ci + 1, :], rg[:])
                    iv = op.tile([1, TS], f32, name=f"iv{cp}")
                    nc.vector.tensor_scalar(iv[:], gmax[cp][:], float(1.0 / 127.0), None,
                                            A_OP.mult)
                    nc.sync.dma_start(bounce[2 + ci:3 + ci, :], iv[:])
                    sr = op.tile([128, TS], f32, name=f"sreps{cp}")
                    nc.sync.dma_start(sr[:], bounce[ci:ci + 1, :].to_broadcast((128, TS)))
                    srep_s[cp] = sr
                    si = op.tile([128, TS], f32, name=f"srepi{cp}")
                    nc.sync.dma_start(si[:], bounce[2 + ci:3 + ci, :].to_broadcast((128, TS)))
                    srep_inv[cp] = si

                # quantize+dequantize (and r+i sum) each 128-row contraction chunk
                # d16 = hfile*8 + c maps to ag[hfile] rows [c*R2 + cp*128 : +128]
                xq = {}
                for d16 in range(16):
                    hfile, c2 = d16 // 8, d16 % 8
                    for ci, cp in enumerate(("r", "i")):
                        r0 = c2 * R2 + ci * 128
                        xt = op.tile([128, TS], f16, name=f"xt{cp}{d16}", tag="xt")
                        nc.sync.dma_start(xt[:], ag[hfile][r0:r0 + 128, :])
                        m = op.tile([128, TS], f32, name=f"xm{cp}{d16}", tag="xm")
                        nc.vector.tensor_tensor(m[:], xt[:], srep_s[cp][:], A_OP.mult)
                        r_ = op.tile([128, TS], f32, name=f"xr_{cp}{d16}", tag="xr_")
                        nc.vector.tensor_scalar(r_[:], m[:], MAGIC, MAGIC, A_OP.add,
                                                A_OP.subtract)
                        d_ = op.tile([128, TS], f16, name=f"xd{cp}{d16}")
                        nc.vector.tensor_tensor(d_[:], r_[:], srep_inv[cp][:], A_OP.mult)
                        xq[(cp, d16)] = d_
                    s_t = op.tile([128, TS], f16, name=f"xsum{d16}")
                    nc.vector.tensor_tensor(s_t[:], xq[("r", d16)][:], xq[("i", d16)][:],
                                            A_OP.add)
                    xq[("s", d16)] = s_t

                for jb in range(4):
                    w1, w2, w3 = wo_pre.pop(jb) if jb in wo_pre else load_wo_jb(jb)
                    pg = {}
                    for tt in range(2):
                        pg[(tt, 1)] = psum_at(3 * tt + 0)
                        pg[(tt, 2)] = psum_at(3 * tt + 1)
                        pg[(tt, 3)] = psum_at(3 * tt + 2)
                    for m, w, cp in ((1, w1, "s"), (2, w2, "r"), (3, w3, "i")):
                        for d16 in range(16):
                            st = (d16 == 0); sp = (d16 == 15)
                            for tt in range(2):
                                lhs = xq[(cp, d16)][:, tt * 128:(tt + 1) * 128]
                                nc.tensor.matmul(pg[(tt, m)][:], lhs, w[:, d16, :],
                                                 start=st, stop=sp)
                    for tt in range(2):
                        c1 = od.tile([128, 512], f32, name=f"oc1{jb}{tt}", tag="oc1")
                        nc.scalar.activation(c1[:], pg[(tt, 1)][:],
                                             mybir.ActivationFunctionType.Copy)
                        fr = od.tile([128, 512], f32, name=f"fr{jb}{tt}", tag="fr")
                        nc.vector.tensor_tensor(fr[:], c1[:], pg[(tt, 2)][:], A_OP.add)
                        nc.sync.dma_start(
                            A["yr_part"][tt * 128:(tt + 1) * 128, jb * 512:(jb + 1) * 512],
                            fr[:])
                        fi = od.tile([128, 512], f32, name=f"fi{jb}{tt}", tag="fi")
                        nc.vector.tensor_tensor(fi[:], c1[:], pg[(tt, 3)][:], A_OP.subtract)
                        nc.sync.dma_start(
                            A["yi_part"][tt * 128:(tt + 1) * 128, jb * 512:(jb + 1) * 512],
                            fi[:])


_CACHE = {}

def _get_compiled():
    if "nc" not in _CACHE:
        from contextlib import ExitStack
        nc, A = build_nc()
        with tile.TileContext(nc) as tc:
            with ExitStack() as ctx:
                emit(nc, A, tc, ctx)
        nc.compile()
        _CACHE["nc"] = nc
    return _CACHE["nc"]


def _host_prep(hidden_real, hidden_imag, positions,
               Wq_r, Wq_i, Wk_r, Wk_i, Wv_r, Wv_i, Wo_r, Wo_i):
    fp16 = np.float16
    f = np.float32
    hr = np.asarray(hidden_real, f); hi = np.asarray(hidden_imag, f)

    def quant(x):
        m = np.maximum(np.abs(x).max(1, keepdims=True), f(1e-5))
        s = (f(127.0) / m).astype(f)
        q = np.clip(np.rint(x * s), -128.0, 127.0).astype(f)
        return (q / s).astype(f)

    qr = quant(hr); qi = quant(hi); qs = qr + qi
    inv_freq = (f(1.0) / (f(10000.0) ** (np.arange(D, dtype=f) / f(D)))).astype(f)
    freqs = np.asarray(positions, np.int32).astype(f)[:, None] * inv_freq[None, :]
    cos = np.cos(freqs).astype(f).astype(ml_dtypes.bfloat16).astype(fp16)
    sin = np.sin(freqs).astype(f).astype(ml_dtypes.bfloat16).astype(fp16)
    col = np.arange(512)[None, :]; row = np.arange(128)[:, None]
    masks = np.concatenate(
        [np.where(col >= 128 * m + row, f(0.0), f(-60000.0)) for m in range(4)],
        axis=1).astype(fp16)

    # o-proj weight rows permuted to the device contraction order:
    # d16 = hfile*8 + c  ->  head (2c + hfile)
    perm = np.empty(H, np.int64)
    for d16 in range(16):
        hfile, c2 = d16 // 8, d16 % 8
        head = 2 * c2 + hfile
        perm[d16 * 128:(d16 + 1) * 128] = np.arange(head * 128, head * 128 + 128)
    Wor = np.asarray(Wo_r, f); Woi = np.asarray(Wo_i, f)
    asc = np.ascontiguousarray
    base = {
        "xr": asc(qr.T.astype(fp16)), "xi": asc(qi.T.astype(fp16)),
        "xs": asc(qs.T.astype(fp16)),
        "wo1": asc(Woi.T[perm].astype(fp16)),
        "wo2": asc((Wor - Woi).T[perm].astype(fp16)),
        "wo3": asc((Wor + Woi).T[perm].astype(fp16)),
        "cosT": asc(cos.T), "sinT": asc(sin.T),
        "masks": masks, "ident": np.eye(128, dtype=fp16),
    }
    in_maps = []
    for c in range(NC):
        sl = slice(c * DS, (c + 1) * DS)
        im = dict(base)
        for nm, Wr_, Wi_ in (("q", Wq_r, Wq_i), ("k", Wk_r, Wk_i), ("v", Wv_r, Wv_i)):
            Wr = np.asarray(Wr_, f)[sl]; Wi = np.asarray(Wi_, f)[sl]
            im[f"w{nm}1"] = asc(Wi.T.astype(fp16))
            im[f"w{nm}2"] = asc((Wr - Wi).T.astype(fp16))
            im[f"w{nm}3"] = asc((Wr + Wi).T.astype(fp16))
        in_maps.append(im)
    return in_maps


def kernel(**inputs):
    nc = _get_compiled()
    in_maps = _host_prep(**inputs)
    res = run_bass_kernel_spmd(nc, in_maps, list(range(NC)))
    yr = np.concatenate([res.results[c]["yr_part"] for c in range(NC)], axis=0)
    yi = np.concatenate([res.results[c]["yi_part"] for c in range(NC)], axis=0)
    return yr, yi


# revision 17
# speedup vs baseline: 1.8071x; 1.8071x over previous
"""nn_ComplexNetAttention on 8 trn2 NeuronCores — v2.

Sharding: heads column-parallel for QKV+attention (2 heads/core), per-head
fp16 AllToAll to redistribute attention output to token-sharded layout
(overlapping head-1 compute), token-parallel o-projection (256 tokens/core).

vs v1:
- int8 fake-quant of hidden computed on HOST; device receives dequantized
  fp16 activations and their r+i sum — removes the device quant phase.
- 3-matmul Karatsuba for all complex projections (QKV and O).
- softmax exp tiles in bf16 (fp16 overflows: max score*scale ~ 14).
- fp16 A2A payload split per head, overlapped with attention compute.
- o-proj weights preloaded / double-buffered ahead of use.
"""
import numpy as np
import ml_dtypes

import concourse.bass as bass
import concourse.bacc as bacc
import concourse.tile as tile
import concourse.mybir as mybir
from concourse.bass_utils import run_bass_kernel_spmd

f32 = mybir.dt.float32
f16 = mybir.dt.float16
bf16 = mybir.dt.bfloat16

T, H, NH, D = 2048, 2048, 16, 128
NC = 8
HPC = NH // NC          # heads per core = 2
DS = HPC * D            # d_out slice per core = 256
TS = T // NC            # tokens per core for o-proj = 256
CH = 256                # token chunk in projection phase
NCH = T // CH           # 8
R2 = 2 * D + 2          # per-src rows in per-head A2A payload = 258
MAGIC = float(2**23 + 2**22)  # fp32 round-to-nearest-even integer trick
A_OP = mybir.AluOpType
HT = H // 128           # 16


def build_nc():
    nc = bacc.Bacc("TRN2", target_bir_lowering=False, debug=False, num_devices=NC)
    A = {}
    def inp(name, shape, dt=f16):
        A[name] = nc.dram_tensor(name, shape, dt, kind="ExternalInput").ap()
    inp("xr", [H, T]); inp("xi", [H, T]); inp("xs", [H, T])
    for t_ in ("q", "k", "v"):
        for m in (1, 2, 3):
            inp(f"w{t_}{m}", [H, DS])
    for m in (1, 2, 3):
        inp(f"wo{m}", [H, H])
    inp("cosT", [D, T]); inp("sinT", [D, T])
    inp("masks", [128, 4 * 512])
    inp("ident", [128, 128])
    A["yr_part"] = nc.dram_tensor("yr_part", [TS, H], f32, kind="ExternalOutput").ap()
    A["yi_part"] = nc.dram_tensor("yi_part", [TS, H], f32, kind="ExternalOutput").ap()
    return nc, A


def _chunked(ap):
    """DRAM [H, w] -> [128, HT, w] view (partition, h-chunk, col)."""
    return ap.rearrange("(a b) c -> b a c", b=128)


def emit(nc, A, tc, ctx):
    const = ctx.enter_context(tc.tile_pool(name="const", bufs=1))
    ps = ctx.enter_context(tc.tile_pool(name="ps", bufs=1, space="PSUM"))
    dram = ctx.enter_context(tc.tile_pool(name="dram", bufs=1, space="DRAM"))

    ident = const.tile([128, 128], f16, name="ident_t")
    nc.sync.dma_start(ident[:], A["ident"][:])
    masks = const.tile([128, 4 * 512], f16, name="masks_t")
    nc.sync.dma_start(masks[:], A["masks"][:])

    # rotating psum allocator over 8 bank tags (max slot shape 128x512 f32)
    _pn = [0]
    def psum(shape=(128, 512)):
        t = ps.tile(list(shape), f32, name=f"pt{_pn[0]}", tag=f"p{_pn[0] % 8}")
        _pn[0] += 1
        return t
    def psum_at(k, shape=(128, 512), dt=f32):
        t = ps.tile(list(shape), dt, name=f"pta{_pn[0]}_{k}", tag=f"p{k % 8}")
        return t

    cont = [dram.tile([NC * R2, TS], f16, name=f"cont{h}") for h in range(HPC)]
    ag = [dram.tile([NC * R2, TS], f16, name=f"ag{h}") for h in range(HPC)]
    bounce = dram.tile([4, TS], f32, name="bounce")

    wo_pool_box = []
    def load_wo_jb(jb):
        ws = []
        for m in (1, 2, 3):
            w = wo_pool_box[0].tile([128, HT, 512], f16, name=f"wo{m}_{jb}", tag=f"wo{m}")
            nc.sync.dma_start(w[:], _chunked(A[f"wo{m}"][:, jb * 512:(jb + 1) * 512]))
            ws.append(w)
        return ws

    with tc.tile_pool(name="qk", bufs=1) as qk_pool:
        qrot = {}
        for tn in ("q", "k"):
            for hd in range(HPC):
                for cp in ("r", "i"):
                    qrot[(tn, hd, cp)] = qk_pool.tile([128, T], f16, name=f"{tn}rot{hd}{cp}")
        vjoin = {}
        for bk in range(T // 128):
            for hd in range(HPC):
                vjoin[(hd, bk)] = qk_pool.tile([128, 257], bf16, name=f"vj{hd}_{bk}")
                nc.vector.memset(vjoin[(hd, bk)][:, 256:257], 1.0)

        # ======== phase 1: QKV projections (Karatsuba), 256-token chunks ======
        with tc.tile_pool(name="wqk", bufs=1) as wqk_pool, \
             tc.tile_pool(name="acts", bufs=2) as acts_pool, \
             tc.tile_pool(name="cmb", bufs=2) as cmb:
            wqk = {}
            for tn in ("q", "k", "v"):
                for m in (1, 2, 3):
                    w = wqk_pool.tile([128, HT, DS], f16, name=f"w{tn}{m}_t")
                    nc.sync.dma_start(w[:], _chunked(A[f"w{tn}{m}"][:]))
                    wqk[(tn, m)] = w
            cosT = wqk_pool.tile([D, T], f16, name="cosT_t")
            nc.sync.dma_start(cosT[:], A["cosT"][:])
            sinT = wqk_pool.tile([D, T], f16, name="sinT_t")
            nc.sync.dma_start(sinT[:], A["sinT"][:])

            for ch in range(NCH):
                t0 = ch * CH
                csl = slice(t0, t0 + CH)
                acts = {}
                for cp in ("r", "i", "s"):
                    a = acts_pool.tile([128, HT, CH], f16, name=f"acts{cp}{ch}",
                                       tag=f"acts{cp}")
                    nc.sync.dma_start(a[:], _chunked(A[f"x{cp}"][:, csl]))
                    acts[cp] = a

                # Q/K: out[d_out 128, tok CH]; M1=Wi*xs, M2=(Wr-Wi)*xr, M3=(Wr+Wi)*xi
                for tn in ("q", "k"):
                    for dt_ in range(HPC):
                        dsl = slice(dt_ * 128, dt_ * 128 + 128)
                        p1, p2, p3 = psum((128, CH)), psum((128, CH)), psum((128, CH))
                        for h in range(HT):
                            st = (h == 0); sp = (h == HT - 1)
                            nc.tensor.matmul(p1[:], wqk[(tn, 1)][:, h, dsl], acts["s"][:, h, :],
                                             start=st, stop=sp)
                        for h in range(HT):
                            st = (h == 0); sp = (h == HT - 1)
                            nc.tensor.matmul(p2[:], wqk[(tn, 2)][:, h, dsl], acts["r"][:, h, :],
                                             start=st, stop=sp)
                        for h in range(HT):
                            st = (h == 0); sp = (h == HT - 1)
                            nc.tensor.matmul(p3[:], wqk[(tn, 3)][:, h, dsl], acts["i"][:, h, :],
                                             start=st, stop=sp)
                        c1 = cmb.tile([128, CH], f32, name=f"c1{tn}{dt_}{ch}", tag="c1")
                        nc.scalar.activation(c1[:], p1[:], mybir.ActivationFunctionType.Copy)
                        y_r = cmb.tile([128, CH], f32, name=f"yr{tn}{dt_}{ch}", tag="yr")
                        y_i = cmb.tile([128, CH], f32, name=f"yi{tn}{dt_}{ch}", tag="yi")
                        nc.vector.tensor_tensor(y_r[:], c1[:], p2[:], A_OP.add)
                        nc.vector.tensor_tensor(y_i[:], c1[:], p3[:], A_OP.subtract)
                        t1 = cmb.tile([128, CH], f32, name=f"t1{tn}{dt_}{ch}", tag="t1")
                        t2 = cmb.tile([128, CH], f32, name=f"t2{tn}{dt_}{ch}", tag="t2")
                        nc.vector.tensor_tensor(t1[:], y_r[:], cosT[:, csl], A_OP.mult)
                        nc.vector.tensor_tensor(t2[:], y_i[:], sinT[:, csl], A_OP.mult)
                        nc.vector.tensor_tensor(qrot[(tn, dt_, "r")][:, csl], t1[:], t2[:],
                                                A_OP.subtract)
                        nc.vector.tensor_tensor(t1[:], y_i[:], cosT[:, csl], A_OP.mult)
                        nc.vector.tensor_tensor(t2[:], y_r[:], sinT[:, csl], A_OP.mult)
                        nc.vector.tensor_tensor(qrot[(tn, dt_, "i")][:, csl], t1[:], t2[:],
                                                A_OP.add)

                # V: out[tok 128, d_out DS]; M1=xs*Vi, M2=xr*(Vr-Vi), M3=xi*(Vr+Vi)
                for tt in range(CH // 128):
                    bk = ch * (CH // 128) + tt
                    tsl = slice(tt * 128, tt * 128 + 128)
                    p1, p2, p3 = psum((128, DS)), psum((128, DS)), psum((128, DS))
                    for h in range(HT):
                        st = (h == 0); sp = (h == HT - 1)
                        nc.tensor.matmul(p1[:], acts["s"][:, h, tsl], wqk[("v", 1)][:, h, :],
                                         start=st, stop=sp)
                    for h in range(HT):
                        st = (h == 0); sp = (h == HT - 1)
                        nc.tensor.matmul(p2[:], acts["r"][:, h, tsl], wqk[("v", 2)][:, h, :],
                                         start=st, stop=sp)
                    for h in range(HT):
                        st = (h == 0); sp = (h == HT - 1)
                        nc.tensor.matmul(p3[:], acts["i"][:, h, tsl], wqk[("v", 3)][:, h, :],
                                         start=st, stop=sp)
                    cv = cmb.tile([128, DS], f32, name=f"cv{ch}{tt}", tag="cv")
                    nc.scalar.activation(cv[:], p1[:], mybir.ActivationFunctionType.Copy)
                    for hd in range(HPC):
                        hsl = slice(hd * 128, hd * 128 + 128)
                        nc.vector.tensor_tensor(vjoin[(hd, bk)][:, 0:128],
                                                cv[:, hsl], p2[:, hsl], A_OP.add)
                        nc.vector.tensor_tensor(vjoin[(hd, bk)][:, 128:256],
                                                cv[:, hsl], p3[:, hsl], A_OP.subtract)

        with tc.tile_pool(name="wo", bufs=2) as wo_pool:
            wo_pool_box.append(wo_pool)
            # preload first o-proj weight block (runs during attention)
            wo_pre = {0: load_wo_jb(0)}

            # ======== phase 2: attention per head + per-head AllToAll ========
            SC = float(1.0 / np.sqrt(2 * D))
            with tc.tile_pool(name="attn", bufs=1) as at, \
                 tc.tile_pool(name="epool", bufs=2) as ep, \
                 tc.tile_pool(name="tp", bufs=2) as tp:
                for hd in range(HPC):
                    out_nat = {}
                    for g in range(4):
                        etiles = {}
                        for bk in range(4 * g + 4):
                            pS = psum_at(bk % 4)
                            qsl = slice(g * 512, g * 512 + 512)
                            nc.tensor.matmul(pS[:],
                                             qrot[("k", hd, "r")][:, bk * 128:bk * 128 + 128],
                                             qrot[("q", hd, "r")][:, qsl],
                                             start=True, stop=False)
                            nc.tensor.matmul(pS[:],
                                             qrot[("k", hd, "i")][:, bk * 128:bk * 128 + 128],
                                             qrot[("q", hd, "i")][:, qsl],
                                             start=False, stop=True)
                            if bk >= 4 * g:
                                mc = (bk - 4 * g) * 512
                                nc.vector.tensor_tensor(pS[:], pS[:], masks[:, mc:mc + 512],
                                                        A_OP.add)
                            e = ep.tile([128, 512], bf16, name=f"e{hd}{g}_{bk}", tag=f"e{bk}")
                            nc.scalar.activation(e[:], pS[:], mybir.ActivationFunctionType.Exp,
                                                 scale=SC)
                            etiles[bk] = e
                        for bq in range(4 * g, 4 * g + 4):
                            pO = psum_at(4 + bq % 4, (128, 257))
                            col = (bq - 4 * g) * 128
                            for bk in range(bq + 1):
                                nc.tensor.matmul(pO[:], etiles[bk][:, col:col + 128],
                                                 vjoin[(hd, bk)][:],
                                                 start=(bk == 0), stop=(bk == bq))
                            rec = at.tile([128, 1], f32, name=f"rec{hd}{bq}")
                            nc.vector.reciprocal(rec[:], pO[:, 256:257])
                            for ci, cp in enumerate(("r", "i")):
                                o = at.tile([128, 128], f16, name=f"on{hd}{cp}{bq}",
                                            tag=f"on{cp}{bq}")
                                nc.vector.tensor_scalar(o[:], pO[:, ci * 128:ci * 128 + 128],
                                                        rec[:], None, A_OP.mult)
                                out_nat[(cp, bq)] = o
                                mx = at.tile([128, 1], f16, name=f"mx{hd}{cp}{bq}")
                                nc.vector.tensor_reduce(mx[:], o[:], mybir.AxisListType.X,
                                                        A_OP.max, apply_absolute_value=True)
                                s_, c0 = bq // 2, (bq % 2) * 128
                                dst = cont[hd][s_ * R2 + 2 * D + ci: s_ * R2 + 2 * D + ci + 1,
                                               c0:c0 + 128]
                                nc.sync.dma_start(dst.rearrange("a b -> b a"), mx[:])

                    # transpose to [d, tok] and scatter into cont[hd]
                    for ci, cp in enumerate(("r", "i")):
                        oT = tp.tile([128, T], f16, name=f"oT{hd}{cp}", tag="oT")
                        for bq in range(T // 128):
                            pT = psum_at(bq % 2, (128, 128), f16)
                            nc.tensor.transpose(pT[:], out_nat[(cp, bq)][:], ident[:])
                            nc.vector.tensor_copy(oT[:, bq * 128:bq * 128 + 128], pT[:])
                        r0 = ci * 128
                        for s_ in range(NC):
                            nc.sync.dma_start(cont[hd][s_ * R2 + r0: s_ * R2 + r0 + 128, :],
                                              oT[:, s_ * TS:(s_ + 1) * TS])
                    nc.gpsimd.collective_compute(
                        "AllToAll", A_OP.bypass, replica_groups=[list(range(NC))],
                        ins=[cont[hd][:].opt()], outs=[ag[hd][:].opt()])

            # ======== phase 3: o-projection on my 256-token slice ========
            wo_pre[1] = load_wo_jb(1)
            with tc.tile_pool(name="op", bufs=1) as op, \
                 tc.tile_pool(name="od", bufs=2) as od:
                # global per-token absmax over 16 (core, head) sources
                gmax = {}
                for ci, cp in enumerate(("r", "i")):
                    g = op.tile([1, TS], f32, name=f"gmax{cp}")
                    first = True
                    for hd in range(HPC):
                        agrows = ag[hd][:].rearrange("(s r) c -> r s c", r=R2)
                        mrows = op.tile([1, NC, TS], f16, name=f"mrows{cp}{hd}", tag="mrows")
                        nc.sync.dma_start(mrows[:], agrows[2 * D + ci: 2 * D + ci + 1, :, :])
                        for s_ in range(NC):
                            if first:
                                nc.vector.tensor_tensor(g[:], mrows[:, s_, :], mrows[:, s_, :],
                                                        A_OP.max)
                                first = False
                            else:
                                nc.vector.tensor_tensor(g[:], g[:], mrows[:, s_, :], A_OP.max)
                    nc.vector.tensor_scalar(g[:], g[:], 1e-5, None, A_OP.max)
                    gmax[cp] = g
                srep_s = {}; srep_inv = {}
                for ci, cp in enumerate(("r", "i")):
                    rg = op.tile([1, TS], f32, name=f"rg{cp}")
                    nc.vector.reciprocal(rg[:], gmax[cp][:])
                    nc.vector.tensor_scalar(rg[:], rg[:], 127.0, None, A_OP.mult)
                    nc.sync.dma_start(bounce[ci:ci + 1, :], rg[:])
                    iv = op.tile([1, TS], f32, name=f"iv{cp}")
                    nc.vector.tensor_scalar(iv[:], gmax[cp][:], float(1.0 / 127.0), None,
                                            A_OP.mult)
                    nc.sync.dma_start(bounce[2 + ci:3 + ci, :], iv[:])
                    sr = op.tile([128, TS], f32, name=f"sreps{cp}")
                    nc.sync.dma_start(sr[:], bounce[ci:ci + 1, :].to_broadcast((128, TS)))
                    srep_s[cp] = sr
                    si = op.tile([128, TS], f32, name=f"srepi{cp}")
                    nc.sync.dma_start(si[:], bounce[2 + ci:3 + ci, :].to_broadcast((128, TS)))
                    srep_inv[cp] = si

                # quantize+dequantize (and r+i sum) each 128-row contraction chunk
                # d16 = hfile*8 + c maps to ag[hfile] rows [c*R2 + cp*128 : +128]
                xq = {}
                for d16 in range(16):
                    hfile, c2 = d16 // 8, d16 % 8
                    for ci, cp in enumerate(("r", "i")):
                        r0 = c2 * R2 + ci * 128
                        xt = op.tile([128, TS], f16, name=f"xt{cp}{d16}", tag="xt")
                        nc.sync.dma_start(xt[:], ag[hfile][r0:r0 + 128, :])
                        m = op.tile([128, TS], f32, name=f"xm{cp}{d16}", tag="xm")
                        nc.vector.tensor_tensor(m[:], xt[:], srep_s[cp][:], A_OP.mult)
                        r_ = op.tile([128, TS], f32, name=f"xr_{cp}{d16}", tag="xr_")
                        nc.vector.tensor_scalar(r_[:], m[:], MAGIC, MAGIC, A_OP.add,
                                                A_OP.subtract)
                        d_ = op.tile([128, TS], f16, name=f"xd{cp}{d16}")
                        nc.vector.tensor_tensor(d_[:], r_[:], srep_inv[cp][:], A_OP.mult)
                        xq[(cp, d16)] = d_
                    s_t = op.tile([128, TS], f16, name=f"xsum{d16}")
                    nc.vector.tensor_tensor(s_t[:], xq[("r", d16)][:], xq[("i", d16)][:],
                                            A_OP.add)
                    xq[("s", d16)] = s_t

                for jb in range(4):
                    w1, w2, w3 = wo_pre.pop(jb) if jb in wo_pre else load_wo_jb(jb)
                    pg = {}
                    for tt in range(2):
                        pg[(tt, 1)] = psum_at(3 * tt + 0)
                        pg[(tt, 2)] = psum_at(3 * tt + 1)
                        pg[(tt, 3)] = psum_at(3 * tt + 2)
                    for m, w, cp in ((1, w1, "s"), (2, w2, "r"), (3, w3, "i")):
                        for d16 in range(16):
                            st = (d16 == 0); sp = (d16 == 15)
                            for tt in range(2):
                                lhs = xq[(cp, d16)][:, tt * 128:(tt + 1) * 128]
                                nc.tensor.matmul(pg[(tt, m)][:], lhs, w[:, d16, :],
                                                 start=st, stop=sp)
                    for tt in range(2):
                        c1 = od.tile([128, 512], f32, name=f"oc1{jb}{tt}", tag="oc1")
                        nc.scalar.activation(c1[:], pg[(tt, 1)][:],
                                             mybir.ActivationFunctionType.Copy)
                        fr = od.tile([128, 512], f32, name=f"fr{jb}{tt}", tag="fr")
                        nc.vector.tensor_tensor(fr[:], c1[:], pg[(tt, 2)][:], A_OP.add)
                        nc.sync.dma_start(
                            A["yr_part"][tt * 128:(tt + 1) * 128, jb * 512:(jb + 1) * 512],
                            fr[:])
                        fi = od.tile([128, 512], f32, name=f"fi{jb}{tt}", tag="fi")
                        nc.vector.tensor_tensor(fi[:], c1[:], pg[(tt, 3)][:], A_OP.subtract)
                        nc.sync.dma_start(
                            A["yi_part"][tt * 128:(tt + 1) * 128, jb * 512:(jb + 1) * 512],
                            fi[:])


_CACHE = {}

def _get_compiled():
    if "nc" not in _CACHE:
        from contextlib import ExitStack
        nc, A = build_nc()
        with tile.TileContext(nc) as tc:
            with ExitStack() as ctx:
                emit(nc, A, tc, ctx)
        nc.compile()
        _CACHE["nc"] = nc
    return _CACHE["nc"]


def _host_prep(hidden_real, hidden_imag, positions,
               Wq_r, Wq_i, Wk_r, Wk_i, Wv_r, Wv_i, Wo_r, Wo_i):
    fp16 = np.float16
    f = np.float32
    hr = np.asarray(hidden_real, f); hi = np.asarray(hidden_imag, f)

    def quant(x):
        m = np.maximum(np.abs(x).max(1, keepdims=True), f(1e-5))
        s = (f(127.0) / m).astype(f)
        q = np.clip(np.rint(x * s), -128.0, 127.0).astype(f)
        return (q / s).astype(f)

    qr = quant(hr); qi = quant(hi); qs = qr + qi
    inv_freq = (f(1.0) / (f(10000.0) ** (np.arange(D, dtype=f) / f(D)))).astype(f)
    freqs = np.asarray(positions, np.int32).astype(f)[:, None] * inv_freq[None, :]
    cos = np.cos(freqs).astype(f).astype(ml_dtypes.bfloat16).astype(fp16)
    sin = np.sin(freqs).astype(f).astype(ml_dtypes.bfloat16).astype(fp16)
    col = np.arange(512)[None, :]; row = np.arange(128)[:, None]
    masks = np.concatenate(
        [np.where(col >= 128 * m + row, f(0.0), f(-60000.0)) for m in range(4)],
        axis=1).astype(fp16)

    # o-proj weight rows permuted to the device contraction order:
    # d16 = hfile*8 + c  ->  head (2c + hfile)
    perm = np.empty(H, np.int64)
    for d16 in range(16):
        hfile, c2 = d16 // 8, d16 % 8
        head = 2 * c2 + hfile
        perm[d16 * 128:(d16 + 1) * 128] = np.arange(head * 128, head * 128 + 128)
    Wor = np.asarray(Wo_r, f); Woi = np.asarray(Wo_i, f)
    asc = np.ascontiguousarray
    base = {
        "xr": asc(qr.T.astype(fp16)), "xi": asc(qi.T.astype(fp16)),
        "xs": asc(qs.T.astype(fp16)),
        "wo1": asc(Woi.T[perm].astype(fp16)),
        "wo2": asc((Wor - Woi).T[perm].astype(fp16)),
        "wo3": asc((Wor + Woi).T[perm].astype(fp16)),
        "cosT": asc(cos.T), "sinT": asc(sin.T),
        "masks": masks, "ident": np.eye(128, dtype=fp16),
    }
    in_maps = []
    for c in range(NC):
        sl = slice(c * DS, (c + 1) * DS)
        im = dict(base)
        for nm, Wr_, Wi_ in (("q", Wq_r, Wq_i), ("k", Wk_r, Wk_i), ("v", Wv_r, Wv_i)):
            Wr = np.asarray(Wr_, f)[sl]; Wi = np.asarray(Wi_, f)[sl]
            im[f"w{nm}1"] = asc(Wi.T.astype(fp16))
            im[f"w{nm}2"] = asc((Wr - Wi).T.astype(fp16))
            im[f"w{nm}3"] = asc((Wr + Wi).T.astype(fp16))
        in_maps.append(im)
    return in_maps


def kernel(**inputs):
    nc = _get_compiled()
    in_maps = _host_prep(**inputs)
    res = run_bass_kernel_spmd(nc, in_maps, list(range(NC)))
    yr = np.concatenate([res.results[c]["yr_part"] for c in range(NC)], axis=0)
    yi = np.concatenate([res.results[c]["yi_part"] for c in range(NC)], axis=0)
    return yr, yi


# revision 19
# speedup vs baseline: 1.8738x; 1.0370x over previous
"""nn_ComplexNetAttention on 8 trn2 NeuronCores — v3.

Sharding: heads column-parallel for QKV+attention (2 heads/core), per-head
fp16 AllToAll to redistribute attention output to token-sharded layout
(overlapping head-1 compute), token-parallel o-projection (256 tokens/core).

- int8 fake-quant of hidden computed on HOST; device receives dequantized
  fp16 activations and their r+i sum — no device quant phase.
- 3-matmul Karatsuba for all complex projections (QKV and O).
- softmax exp tiles in bf16 (fp16 overflows: max score*scale ~ 14).
- causal mask via 0/1 multiply on the diagonal 128x128 e-slice only.
- attention software-pipelined: QK(g) overlaps pO(g-1).
- all large DMAs from host-pre-chunked contiguous layouts.
- o-proj weights preloaded / double-buffered ahead of use.
"""
import numpy as np
import ml_dtypes

import concourse.bass as bass
import concourse.bacc as bacc
import concourse.tile as tile
import concourse.mybir as mybir
from concourse.bass_utils import run_bass_kernel_spmd

f32 = mybir.dt.float32
f16 = mybir.dt.float16
bf16 = mybir.dt.bfloat16

T, H, NH, D = 2048, 2048, 16, 128
NC = 8
HPC = NH // NC          # heads per core = 2
DS = HPC * D            # d_out slice per core = 256
TS = T // NC            # tokens per core for o-proj = 256
CH = 256                # token chunk in projection phase
NCH = T // CH           # 8
R2 = 2 * D + 2          # per-src rows in per-head A2A payload = 258
MAGIC = float(2**23 + 2**22)  # fp32 round-to-nearest-even integer trick
A_OP = mybir.AluOpType
HT = H // 128           # 16


def build_nc():
    nc = bacc.Bacc("TRN2", target_bir_lowering=False, debug=False, num_devices=NC)
    A = {}
    def inp(name, shape, dt=f16):
        A[name] = nc.dram_tensor(name, shape, dt, kind="ExternalInput").ap()
    # pre-chunked: x*[ch*128+p, h*CH+t], w{q,k,v}*[p, h*DS+j], wo*[jb*128+p, h*512+j]
    inp("xr", [NCH * 128, HT * CH]); inp("xi", [NCH * 128, HT * CH])
    inp("xs", [NCH * 128, HT * CH])
    for t_ in ("q", "k", "v"):
        for m in (1, 2, 3):
            inp(f"w{t_}{m}", [128, HT * DS])
    for m in (1, 2, 3):
        inp(f"wo{m}", [4 * 128, HT * 512])
    inp("cosT", [D, T]); inp("sinT", [D, T])
    inp("tri", [128, 128])
    inp("ident", [128, 128])
    A["yr_part"] = nc.dram_tensor("yr_part", [TS, H], f32, kind="ExternalOutput").ap()
    A["yi_part"] = nc.dram_tensor("yi_part", [TS, H], f32, kind="ExternalOutput").ap()
    return nc, A


def emit(nc, A, tc, ctx):
    const = ctx.enter_context(tc.tile_pool(name="const", bufs=1))
    ps = ctx.enter_context(tc.tile_pool(name="ps", bufs=1, space="PSUM"))
    dram = ctx.enter_context(tc.tile_pool(name="dram", bufs=1, space="DRAM"))

    ident = const.tile([128, 128], f16, name="ident_t")
    nc.sync.dma_start(ident[:], A["ident"][:])
    tri = const.tile([128, 128], f16, name="tri_t")
    nc.sync.dma_start(tri[:], A["tri"][:])

    _pn = [0]
    def psum(shape=(128, 512)):
        t = ps.tile(list(shape), f32, name=f"pt{_pn[0]}", tag=f"p{_pn[0] % 8}")
        _pn[0] += 1
        return t
    def psum_at(k, shape=(128, 512), dt=f32):
        t = ps.tile(list(shape), dt, name=f"pta{_pn[0]}_{k}", tag=f"p{k % 8}")
        return t

    cont = [dram.tile([NC * R2, TS], f16, name=f"cont{h}") for h in range(HPC)]
    ag = [dram.tile([NC * R2, TS], f16, name=f"ag{h}") for h in range(HPC)]
    bounce = dram.tile([4, TS], f32, name="bounce")

    wo_pool_box = []
    def load_wo_jb(jb):
        ws = []
        for m in (1, 2, 3):
            w = wo_pool_box[0].tile([128, HT, 512], f16, name=f"wo{m}_{jb}", tag=f"wo{m}")
            nc.sync.dma_start(
                w[:], A[f"wo{m}"][jb * 128:(jb + 1) * 128, :]
                .rearrange("p (h w) -> p h w", h=HT))
            ws.append(w)
        return ws

    with tc.tile_pool(name="qk", bufs=1) as qk_pool:
        qrot = {}
        for tn in ("q", "k"):
            for hd in range(HPC):
                for cp in ("r", "i"):
                    qrot[(tn, hd, cp)] = qk_pool.tile([128, T], f16, name=f"{tn}rot{hd}{cp}")
        vjoin = {}
        for bk in range(T // 128):
            for hd in range(HPC):
                vjoin[(hd, bk)] = qk_pool.tile([128, 257], bf16, name=f"vj{hd}_{bk}")
                nc.vector.memset(vjoin[(hd, bk)][:, 256:257], 1.0)

        # ======== phase 1: QKV projections (Karatsuba), 256-token chunks ======
        with tc.tile_pool(name="wqk", bufs=1) as wqk_pool, \
             tc.tile_pool(name="acts", bufs=2) as acts_pool, \
             tc.tile_pool(name="cmb", bufs=2) as cmb:

            def load_acts(ch):
                out = {}
                for cp in ("r", "i", "s"):
                    a = acts_pool.tile([128, HT, CH], f16, name=f"acts{cp}{ch}",
                                       tag=f"acts{cp}")
                    nc.sync.dma_start(
                        a[:], A[f"x{cp}"][ch * 128:(ch + 1) * 128, :]
                        .rearrange("p (h w) -> p h w", h=HT))
                    out[cp] = a
                return out

            acts_cache = {0: load_acts(0)}
            wqk = {}
            for tn in ("q", "k", "v"):
                for m in (1, 2, 3):
                    w = wqk_pool.tile([128, HT, DS], f16, name=f"w{tn}{m}_t")
                    nc.sync.dma_start(
                        w[:], A[f"w{tn}{m}"][:].rearrange("p (h w) -> p h w", h=HT))
                    wqk[(tn, m)] = w
            cosT = wqk_pool.tile([D, T], f16, name="cosT_t")
            nc.sync.dma_start(cosT[:], A["cosT"][:])
            sinT = wqk_pool.tile([D, T], f16, name="sinT_t")
            nc.sync.dma_start(sinT[:], A["sinT"][:])

            for ch in range(NCH):
                t0 = ch * CH
                csl = slice(t0, t0 + CH)
                acts = acts_cache.pop(ch)
                if ch + 1 < NCH:
                    acts_cache[ch + 1] = load_acts(ch + 1)

                # Q/K: out[d_out 128, tok CH]; M1=Wi*xs, M2=(Wr-Wi)*xr, M3=(Wr+Wi)*xi
                for tn in ("q", "k"):
                    for dt_ in range(HPC):
                        dsl = slice(dt_ * 128, dt_ * 128 + 128)
                        p1, p2, p3 = psum((128, CH)), psum((128, CH)), psum((128, CH))
                        for h in range(HT):
                            st = (h == 0); sp = (h == HT - 1)
                            nc.tensor.matmul(p1[:], wqk[(tn, 1)][:, h, dsl], acts["s"][:, h, :],
                                             start=st, stop=sp)
                        for h in range(HT):
                            st = (h == 0); sp = (h == HT - 1)
                            nc.tensor.matmul(p2[:], wqk[(tn, 2)][:, h, dsl], acts["r"][:, h, :],
                                             start=st, stop=sp)
                        for h in range(HT):
                            st = (h == 0); sp = (h == HT - 1)
                            nc.tensor.matmul(p3[:], wqk[(tn, 3)][:, h, dsl], acts["i"][:, h, :],
                                             start=st, stop=sp)
                        c1 = cmb.tile([128, CH], f32, name=f"c1{tn}{dt_}{ch}", tag="c1")
                        nc.scalar.activation(c1[:], p1[:], mybir.ActivationFunctionType.Copy)
                        y_r = cmb.tile([128, CH], f32, name=f"yr{tn}{dt_}{ch}", tag="yr")
                        y_i = cmb.tile([128, CH], f32, name=f"yi{tn}{dt_}{ch}", tag="yi")
                        nc.vector.tensor_tensor(y_r[:], c1[:], p2[:], A_OP.add)
                        nc.vector.tensor_tensor(y_i[:], c1[:], p3[:], A_OP.subtract)
                        t1 = cmb.tile([128, CH], f32, name=f"t1{tn}{dt_}{ch}", tag="t1")
                        t2 = cmb.tile([128, CH], f32, name=f"t2{tn}{dt_}{ch}", tag="t2")
                        nc.vector.tensor_tensor(t1[:], y_r[:], cosT[:, csl], A_OP.mult)
                        nc.vector.tensor_tensor(t2[:], y_i[:], sinT[:, csl], A_OP.mult)
                        nc.vector.tensor_tensor(qrot[(tn, dt_, "r")][:, csl], t1[:], t2[:],
                                                A_OP.subtract)
                        nc.vector.tensor_tensor(t1[:], y_i[:], cosT[:, csl], A_OP.mult)
                        nc.vector.tensor_tensor(t2[:], y_r[:], sinT[:, csl], A_OP.mult)
                        nc.vector.tensor_tensor(qrot[(tn, dt_, "i")][:, csl], t1[:], t2[:],
                                                A_OP.add)

                # V: out[tok 128, d_out DS]; M1=xs*Vi, M2=xr*(Vr-Vi), M3=xi*(Vr+Vi)
                for tt in range(CH // 128):
                    bk = ch * (CH // 128) + tt
                    tsl = slice(tt * 128, tt * 128 + 128)
                    p1, p2, p3 = psum((128, DS)), psum((128, DS)), psum((128, DS))
                    for h in range(HT):
                        st = (h == 0); sp = (h == HT - 1)
                        nc.tensor.matmul(p1[:], acts["s"][:, h, tsl], wqk[("v", 1)][:, h, :],
                                         start=st, stop=sp)
                    for h in range(HT):
                        st = (h == 0); sp = (h == HT - 1)
                        nc.tensor.matmul(p2[:], acts["r"][:, h, tsl], wqk[("v", 2)][:, h, :],
                                         start=st, stop=sp)
                    for h in range(HT):
                        st = (h == 0); sp = (h == HT - 1)
                        nc.tensor.matmul(p3[:], acts["i"][:, h, tsl], wqk[("v", 3)][:, h, :],
                                         start=st, stop=sp)
                    cv = cmb.tile([128, DS], f32, name=f"cv{ch}{tt}", tag="cv")
                    nc.scalar.activation(cv[:], p1[:], mybir.ActivationFunctionType.Copy)
                    for hd in range(HPC):
                        hsl = slice(hd * 128, hd * 128 + 128)
                        nc.vector.tensor_tensor(vjoin[(hd, bk)][:, 0:128],
                                                cv[:, hsl], p2[:, hsl], A_OP.add)
                        nc.vector.tensor_tensor(vjoin[(hd, bk)][:, 128:256],
                                                cv[:, hsl], p3[:, hsl], A_OP.subtract)

        with tc.tile_pool(name="wo", bufs=2) as wo_pool:
            wo_pool_box.append(wo_pool)
            # preload first o-proj weight block (runs during attention)
            wo_pre = {0: load_wo_jb(0)}

            # ======== phase 2: attention per head + per-head AllToAll ========
            SC = float(1.0 / np.sqrt(2 * D))
            with tc.tile_pool(name="attn", bufs=1) as at, \
                 tc.tile_pool(name="epool", bufs=2) as ep, \
                 tc.tile_pool(name="tp", bufs=2) as tp:
                for hd in range(HPC):
                    out_nat = {}

                    def emit_qk(g):
                        ets = {}
                        for bk in range(4 * g + 4):
                            pS = psum_at(bk % 4)
                            qsl = slice(g * 512, g * 512 + 512)
                            nc.tensor.matmul(pS[:],
                                             qrot[("k", hd, "r")][:, bk * 128:bk * 128 + 128],
                                             qrot[("q", hd, "r")][:, qsl],
                                             start=True, stop=False)
                            nc.tensor.matmul(pS[:],
                                             qrot[("k", hd, "i")][:, bk * 128:bk * 128 + 128],
                                             qrot[("q", hd, "i")][:, qsl],
                                             start=False, stop=True)
                            e = ep.tile([128, 512], bf16, name=f"e{hd}{g}_{bk}", tag=f"e{bk}")
                            nc.scalar.activation(e[:], pS[:],
                                                 mybir.ActivationFunctionType.Exp, scale=SC)
                            if bk >= 4 * g:
                                # diagonal block: cols < c0 invalid, triangle at c0
                                c0 = (bk - 4 * g) * 128
                                nc.vector.tensor_tensor(e[:, c0:c0 + 128], e[:, c0:c0 + 128],
                                                        tri[:], A_OP.mult)
                                if c0 > 0:
                                    nc.vector.memset(e[:, 0:c0], 0.0)
                            ets[bk] = e
                        return ets

                    def emit_po(g, ets):
                        for bq in range(4 * g, 4 * g + 4):
                            pO = psum_at(4 + bq % 4, (128, 257))
                            col = (bq - 4 * g) * 128
                            for bk in range(bq + 1):
                                nc.tensor.matmul(pO[:], ets[bk][:, col:col + 128],
                                                 vjoin[(hd, bk)][:],
                                                 start=(bk == 0), stop=(bk == bq))
                            rec = at.tile([128, 1], f32, name=f"rec{hd}{bq}")
                            nc.vector.reciprocal(rec[:], pO[:, 256:257])
                            for ci, cp in enumerate(("r", "i")):
                                o = at.tile([128, 128], f16, name=f"on{hd}{cp}{bq}",
                                            tag=f"on{cp}{bq}")
                                nc.vector.tensor_scalar(o[:], pO[:, ci * 128:ci * 128 + 128],
                                                        rec[:], None, A_OP.mult)
                                out_nat[(cp, bq)] = o
                                mx = at.tile([128, 1], f16, name=f"mx{hd}{cp}{bq}")
                                nc.vector.tensor_reduce(mx[:], o[:], mybir.AxisListType.X,
                                                        A_OP.max, apply_absolute_value=True)
                                s_, c0 = bq // 2, (bq % 2) * 128
                                dst = cont[hd][s_ * R2 + 2 * D + ci: s_ * R2 + 2 * D + ci + 1,
                                               c0:c0 + 128]
                                nc.sync.dma_start(dst.rearrange("a b -> b a"), mx[:])

                    prev = None
                    for g in range(4):
                        ets = emit_qk(g)
                        if prev is not None:
                            emit_po(g - 1, prev)
                        prev = ets
                    emit_po(3, prev)

                    # transpose to [d, tok] and scatter into cont[hd]
                    for ci, cp in enumerate(("r", "i")):
                        oT = tp.tile([128, T], f16, name=f"oT{hd}{cp}", tag="oT")
                        for bq in range(T // 128):
                            pT = psum_at(bq % 2, (128, 128), f16)
                            nc.tensor.transpose(pT[:], out_nat[(cp, bq)][:], ident[:])
                            nc.vector.tensor_copy(oT[:, bq * 128:bq * 128 + 128], pT[:])
                        r0 = ci * 128
                        for s_ in range(NC):
                            nc.sync.dma_start(cont[hd][s_ * R2 + r0: s_ * R2 + r0 + 128, :],
                                              oT[:, s_ * TS:(s_ + 1) * TS])
                    nc.gpsimd.collective_compute(
                        "AllToAll", A_OP.bypass, replica_groups=[list(range(NC))],
                        ins=[cont[hd][:].opt()], outs=[ag[hd][:].opt()])

            # ======== phase 3: o-projection on my 256-token slice ========
            wo_pre[1] = load_wo_jb(1)
            with tc.tile_pool(name="op", bufs=1) as op, \
                 tc.tile_pool(name="od", bufs=2) as od:
                # global per-token absmax over 16 (core, head) sources
                gmax = {}
                for ci, cp in enumerate(("r", "i")):
                    g = op.tile([1, TS], f32, name=f"gmax{cp}")
                    first = True
                    for hd in range(HPC):
                        agrows = ag[hd][:].rearrange("(s r) c -> r s c", r=R2)
                        mrows = op.tile([1, NC, TS], f16, name=f"mrows{cp}{hd}", tag="mrows")
                        nc.sync.dma_start(mrows[:], agrows[2 * D + ci: 2 * D + ci + 1, :, :])
                        for s_ in range(NC):
                            if first:
                                nc.vector.tensor_tensor(g[:], mrows[:, s_, :], mrows[:, s_, :],
                                                        A_OP.max)
                                first = False
                            else:
                                nc.vector.tensor_tensor(g[:], g[:], mrows[:, s_, :], A_OP.max)
                    nc.vector.tensor_scalar(g[:], g[:], 1e-5, None, A_OP.max)
                    gmax[cp] = g
                srep_s = {}; srep_inv = {}
                for ci, cp in enumerate(("r", "i")):
                    rg = op.tile([1, TS], f32, name=f"rg{cp}")
                    nc.vector.reciprocal(rg[:], gmax[cp][:])
                    nc.vector.tensor_scalar(rg[:], rg[:], 127.0, None, A_OP.mult)
                    nc.sync.dma_start(bounce[ci:ci + 1, :], rg[:])
                    iv = op.tile([1, TS], f32, name=f"iv{cp}")
                    nc.vector.tensor_scalar(iv[:], gmax[cp][:], float(1.0 / 127.0), None,
                                            A_OP.mult)
                    nc.sync.dma_start(bounce[2 + ci:3 + ci, :], iv[:])
                    sr = op.tile([128, TS], f32, name=f"sreps{cp}")
                    nc.sync.dma_start(sr[:], bounce[ci:ci + 1, :].to_broadcast((128, TS)))
                    srep_s[cp] = sr
                    si = op.tile([128, TS], f32, name=f"srepi{cp}")
                    nc.sync.dma_start(si[:], bounce[2 + ci:3 + ci, :].to_broadcast((128, TS)))
                    srep_inv[cp] = si

                # quantize+dequantize (and r+i sum) each 128-row contraction chunk
                # d16 = hfile*8 + c maps to ag[hfile] rows [c*R2 + cp*128 : +128]
                xq = {}
                for d16 in range(16):
                    hfile, c2 = d16 // 8, d16 % 8
                    for ci, cp in enumerate(("r", "i")):
                        r0 = c2 * R2 + ci * 128
                        xt = op.tile([128, TS], f16, name=f"xt{cp}{d16}", tag="xt")
                        nc.sync.dma_start(xt[:], ag[hfile][r0:r0 + 128, :])
                        m = op.tile([128, TS], f32, name=f"xm{cp}{d16}", tag="xm")
                        nc.vector.tensor_tensor(m[:], xt[:], srep_s[cp][:], A_OP.mult)
                        r_ = op.tile([128, TS], f32, name=f"xr_{cp}{d16}", tag="xr_")
                        nc.vector.tensor_scalar(r_[:], m[:], MAGIC, MAGIC, A_OP.add,
                                                A_OP.subtract)
                        d_ = op.tile([128, TS], f16, name=f"xd{cp}{d16}")
                        nc.vector.tensor_tensor(d_[:], r_[:], srep_inv[cp][:], A_OP.mult)
                        xq[(cp, d16)] = d_
                    s_t = op.tile([128, TS], f16, name=f"xsum{d16}")
                    nc.vector.tensor_tensor(s_t[:], xq[("r", d16)][:], xq[("i", d16)][:],
                                            A_OP.add)
                    xq[("s", d16)] = s_t

                for jb in range(4):
                    w1, w2, w3 = wo_pre.pop(jb) if jb in wo_pre else load_wo_jb(jb)
                    pg = {}
                    for tt in range(2):
                        pg[(tt, 1)] = psum_at(3 * tt + 0)
                        pg[(tt, 2)] = psum_at(3 * tt + 1)
                        pg[(tt, 3)] = psum_at(3 * tt + 2)
                    for m, w, cp in ((1, w1, "s"), (2, w2, "r"), (3, w3, "i")):
                        for d16 in range(16):
                            st = (d16 == 0); sp = (d16 == 15)
                            for tt in range(2):
                                lhs = xq[(cp, d16)][:, tt * 128:(tt + 1) * 128]
                                nc.tensor.matmul(pg[(tt, m)][:], lhs, w[:, d16, :],
                                                 start=st, stop=sp)
                    for tt in range(2):
                        c1 = od.tile([128, 512], f32, name=f"oc1{jb}{tt}", tag="oc1")
                        nc.scalar.activation(c1[:], pg[(tt, 1)][:],
                                             mybir.ActivationFunctionType.Copy)
                        fr = od.tile([128, 512], f32, name=f"fr{jb}{tt}", tag="fr")
                        nc.vector.tensor_tensor(fr[:], c1[:], pg[(tt, 2)][:], A_OP.add)
                        nc.sync.dma_start(
                            A["yr_part"][tt * 128:(tt + 1) * 128, jb * 512:(jb + 1) * 512],
                            fr[:])
                        fi = od.tile([128, 512], f32, name=f"fi{jb}{tt}", tag="fi")
                        nc.vector.tensor_tensor(fi[:], c1[:], pg[(tt, 3)][:], A_OP.subtract)
                        nc.sync.dma_start(
                            A["yi_part"][tt * 128:(tt + 1) * 128, jb * 512:(jb + 1) * 512],
                            fi[:])


_CACHE = {}

def _get_compiled():
    if "nc" not in _CACHE:
        from contextlib import ExitStack
        nc, A = build_nc()
        with tile.TileContext(nc) as tc:
            with ExitStack() as ctx:
                emit(nc, A, tc, ctx)
        nc.compile()
        _CACHE["nc"] = nc
    return _CACHE["nc"]


def _prechunk(x, w):
    """[Hrows, Wcols] -> [Hrows/128 * 128, (Hrows/128-major) cols] contiguous blocks.

    out[blk*128 + p, h*w + j] = x[h*128 + p, blk*w + j]  -- wait, no: blocks over
    COLUMN windows of width w; h iterates row-chunks.
    """
    rows, cols = x.shape
    nb = cols // w
    hh = rows // 128
    out = np.empty((nb * 128, hh * w), dtype=x.dtype)
    xr = x.reshape(hh, 128, nb, w)
    for b in range(nb):
        out[b * 128:(b + 1) * 128, :] = (
            xr[:, :, b, :].transpose(1, 0, 2).reshape(128, hh * w))
    return out


def _host_prep(hidden_real, hidden_imag, positions,
               Wq_r, Wq_i, Wk_r, Wk_i, Wv_r, Wv_i, Wo_r, Wo_i):
    fp16 = np.float16
    f = np.float32
    hr = np.asarray(hidden_real, f); hi = np.asarray(hidden_imag, f)

    def quant(x):
        m = np.maximum(np.abs(x).max(1, keepdims=True), f(1e-5))
        s = (f(127.0) / m).astype(f)
        q = np.clip(np.rint(x * s), -128.0, 127.0).astype(f)
        return (q / s).astype(f)

    qr = quant(hr); qi = quant(hi); qs = qr + qi
    inv_freq = (f(1.0) / (f(10000.0) ** (np.arange(D, dtype=f) / f(D)))).astype(f)
    freqs = np.asarray(positions, np.int32).astype(f)[:, None] * inv_freq[None, :]
    cos = np.cos(freqs).astype(f).astype(ml_dtypes.bfloat16).astype(fp16)
    sin = np.sin(freqs).astype(f).astype(ml_dtypes.bfloat16).astype(fp16)
    tri = (np.arange(128)[None, :] >= np.arange(128)[:, None]).astype(fp16)

    # o-proj weight rows permuted to the device contraction order:
    # d16 = hfile*8 + c  ->  head (2c + hfile)
    perm = np.empty(H, np.int64)
    for d16 in range(16):
        hfile, c2 = d16 // 8, d16 % 8
        head = 2 * c2 + hfile
        perm[d16 * 128:(d16 + 1) * 128] = np.arange(head * 128, head * 128 + 128)
    Wor = np.asarray(Wo_r, f); Woi = np.asarray(Wo_i, f)
    asc = np.ascontiguousarray
    base = {
        "xr": _prechunk(qr.T.astype(fp16), CH),
        "xi": _prechunk(qi.T.astype(fp16), CH),
        "xs": _prechunk(qs.T.astype(fp16), CH),
        "wo1": _prechunk(Woi.T[perm].astype(fp16), 512),
        "wo2": _prechunk((Wor - Woi).T[perm].astype(fp16), 512),
        "wo3": _prechunk((Wor + Woi).T[perm].astype(fp16), 512),
        "cosT": asc(cos.T), "sinT": asc(sin.T),
        "tri": tri, "ident": np.eye(128, dtype=fp16),
    }
    in_maps = []
    for c in range(NC):
        sl = slice(c * DS, (c + 1) * DS)
        im = dict(base)
        for nm, Wr_, Wi_ in (("q", Wq_r, Wq_i), ("k", Wk_r, Wk_i), ("v", Wv_r, Wv_i)):
            Wr = np.asarray(Wr_, f)[sl]; Wi = np.asarray(Wi_, f)[sl]
            im[f"w{nm}1"] = _prechunk(Wi.T.astype(fp16), DS)
            im[f"w{nm}2"] = _prechunk((Wr - Wi).T.astype(fp16), DS)
            im[f"w{nm}3"] = _prechunk((Wr + Wi).T.astype(fp16), DS)
        in_maps.append(im)
    return in_maps


def kernel(**inputs):
    nc = _get_compiled()
    in_maps = _host_prep(**inputs)
    res = run_bass_kernel_spmd(nc, in_maps, list(range(NC)))
    yr = np.concatenate([res.results[c]["yr_part"] for c in range(NC)], axis=0)
    yi = np.concatenate([res.results[c]["yi_part"] for c in range(NC)], axis=0)
    return yr, yi


# revision 20
# speedup vs baseline: 2.0649x; 1.1020x over previous
"""nn_ComplexNetAttention on 8 trn2 NeuronCores — v3.

Sharding: heads column-parallel for QKV+attention (2 heads/core), per-head
fp16 AllToAll to redistribute attention output to token-sharded layout
(overlapping head-1 compute), token-parallel o-projection (256 tokens/core).

- int8 fake-quant of hidden computed on HOST; device receives dequantized
  fp16 activations and their r+i sum — no device quant phase.
- 3-matmul Karatsuba for all complex projections (QKV and O).
- softmax exp tiles in bf16 (fp16 overflows: max score*scale ~ 14).
- causal mask via 0/1 multiply on the diagonal 128x128 e-slice only.
- attention software-pipelined: QK(g) overlaps pO(g-1).
- all large DMAs from host-pre-chunked contiguous layouts.
- o-proj weights preloaded / double-buffered ahead of use.
"""
import numpy as np
import ml_dtypes

import concourse.bass as bass
import concourse.bacc as bacc
import concourse.tile as tile
import concourse.mybir as mybir
from concourse.bass_utils import run_bass_kernel_spmd

f32 = mybir.dt.float32
f16 = mybir.dt.float16
bf16 = mybir.dt.bfloat16

T, H, NH, D = 2048, 2048, 16, 128
NC = 8
HPC = NH // NC          # heads per core = 2
DS = HPC * D            # d_out slice per core = 256
TS = T // NC            # tokens per core for o-proj = 256
CH = 256                # token chunk in projection phase
NCH = T // CH           # 8
R2 = 2 * D + 2          # per-src rows in per-head A2A payload = 258
MAGIC = float(2**23 + 2**22)  # fp32 round-to-nearest-even integer trick
A_OP = mybir.AluOpType
HT = H // 128           # 16


def build_nc():
    nc = bacc.Bacc("TRN2", target_bir_lowering=False, debug=False, num_devices=NC)
    A = {}
    def inp(name, shape, dt=f16):
        A[name] = nc.dram_tensor(name, shape, dt, kind="ExternalInput").ap()
    # pre-chunked: x*[ch*128+p, h*CH+t], w{q,k,v}*[p, h*DS+j], wo*[jb*128+p, h*512+j]
    inp("xr", [NCH * 128, HT * CH]); inp("xi", [NCH * 128, HT * CH])
    inp("xs", [NCH * 128, HT * CH])
    for t_ in ("q", "k", "v"):
        for m in (1, 2, 3):
            inp(f"w{t_}{m}", [128, HT * DS])
    for m in (1, 2, 3):
        inp(f"wo{m}", [4 * 128, HT * 512])
    inp("cosT", [D, T]); inp("sinT", [D, T])
    inp("tri", [128, 128])
    inp("ident", [128, 128])
    A["yr_part"] = nc.dram_tensor("yr_part", [TS, H], f32, kind="ExternalOutput").ap()
    A["yi_part"] = nc.dram_tensor("yi_part", [TS, H], f32, kind="ExternalOutput").ap()
    return nc, A


def emit(nc, A, tc, ctx):
    const = ctx.enter_context(tc.tile_pool(name="const", bufs=1))
    ps = ctx.enter_context(tc.tile_pool(name="ps", bufs=1, space="PSUM"))
    dram = ctx.enter_context(tc.tile_pool(name="dram", bufs=1, space="DRAM"))

    ident = const.tile([128, 128], f16, name="ident_t")
    nc.sync.dma_start(ident[:], A["ident"][:])
    tri = const.tile([128, 128], f16, name="tri_t")
    nc.sync.dma_start(tri[:], A["tri"][:])

    _pn = [0]
    def psum(shape=(128, 512)):
        t = ps.tile(list(shape), f32, name=f"pt{_pn[0]}", tag=f"p{_pn[0] % 8}")
        _pn[0] += 1
        return t
    def psum_at(k, shape=(128, 512), dt=f32):
        t = ps.tile(list(shape), dt, name=f"pta{_pn[0]}_{k}", tag=f"p{k % 8}")
        return t

    cont = dram.tile([NC * HPC * R2, TS], f16, name="cont")
    ag = dram.tile([NC * HPC * R2, TS], f16, name="ag")
    bounce = dram.tile([4, TS], f32, name="bounce")

    wo_pool_box = []
    def load_wo_jb(jb):
        ws = []
        for m in (1, 2, 3):
            w = wo_pool_box[0].tile([128, HT, 512], f16, name=f"wo{m}_{jb}", tag=f"wo{m}")
            nc.sync.dma_start(
                w[:], A[f"wo{m}"][jb * 128:(jb + 1) * 128, :]
                .rearrange("p (h w) -> p h w", h=HT))
            ws.append(w)
        return ws

    with tc.tile_pool(name="qk", bufs=1) as qk_pool:
        qrot = {}
        for tn in ("q", "k"):
            for hd in range(HPC):
                for cp in ("r", "i"):
                    qrot[(tn, hd, cp)] = qk_pool.tile([128, T], f16, name=f"{tn}rot{hd}{cp}")
        vjoin = {}
        for bk in range(T // 128):
            for hd in range(HPC):
                vjoin[(hd, bk)] = qk_pool.tile([128, 257], bf16, name=f"vj{hd}_{bk}")
                nc.vector.memset(vjoin[(hd, bk)][:, 256:257], 1.0)

        # ======== phase 1: QKV projections (Karatsuba), 256-token chunks ======
        with tc.tile_pool(name="wqk", bufs=1) as wqk_pool, \
             tc.tile_pool(name="acts", bufs=2) as acts_pool, \
             tc.tile_pool(name="cmb", bufs=2) as cmb:

            def load_acts(ch):
                out = {}
                for cp in ("r", "i", "s"):
                    a = acts_pool.tile([128, HT, CH], f16, name=f"acts{cp}{ch}",
                                       tag=f"acts{cp}")
                    nc.sync.dma_start(
                        a[:], A[f"x{cp}"][ch * 128:(ch + 1) * 128, :]
                        .rearrange("p (h w) -> p h w", h=HT))
                    out[cp] = a
                return out

            acts_cache = {0: load_acts(0)}
            wqk = {}
            for tn in ("q", "k", "v"):
                for m in (1, 2, 3):
                    w = wqk_pool.tile([128, HT, DS], f16, name=f"w{tn}{m}_t")
                    nc.sync.dma_start(
                        w[:], A[f"w{tn}{m}"][:].rearrange("p (h w) -> p h w", h=HT))
                    wqk[(tn, m)] = w
            cosT = wqk_pool.tile([D, T], f16, name="cosT_t")
            nc.sync.dma_start(cosT[:], A["cosT"][:])
            sinT = wqk_pool.tile([D, T], f16, name="sinT_t")
            nc.sync.dma_start(sinT[:], A["sinT"][:])

            for ch in range(NCH):
                t0 = ch * CH
                csl = slice(t0, t0 + CH)
                acts = acts_cache.pop(ch)
                if ch + 1 < NCH:
                    acts_cache[ch + 1] = load_acts(ch + 1)

                # Q/K: out[d_out 128, tok CH]; M1=Wi*xs, M2=(Wr-Wi)*xr, M3=(Wr+Wi)*xi
                for tn in ("q", "k"):
                    for dt_ in range(HPC):
                        dsl = slice(dt_ * 128, dt_ * 128 + 128)
                        p1, p2, p3 = psum((128, CH)), psum((128, CH)), psum((128, CH))
                        for h in range(HT):
                            st = (h == 0); sp = (h == HT - 1)
                            nc.tensor.matmul(p1[:], wqk[(tn, 1)][:, h, dsl], acts["s"][:, h, :],
                                             start=st, stop=sp)
                        for h in range(HT):
                            st = (h == 0); sp = (h == HT - 1)
                            nc.tensor.matmul(p2[:], wqk[(tn, 2)][:, h, dsl], acts["r"][:, h, :],
                                             start=st, stop=sp)
                        for h in range(HT):
                            st = (h == 0); sp = (h == HT - 1)
                            nc.tensor.matmul(p3[:], wqk[(tn, 3)][:, h, dsl], acts["i"][:, h, :],
                                             start=st, stop=sp)
                        c1 = cmb.tile([128, CH], f32, name=f"c1{tn}{dt_}{ch}", tag="c1")
                        nc.scalar.activation(c1[:], p1[:], mybir.ActivationFunctionType.Copy)
                        y_r = cmb.tile([128, CH], f32, name=f"yr{tn}{dt_}{ch}", tag="yr")
                        y_i = cmb.tile([128, CH], f32, name=f"yi{tn}{dt_}{ch}", tag="yi")
                        nc.vector.tensor_tensor(y_r[:], c1[:], p2[:], A_OP.add)
                        nc.vector.tensor_tensor(y_i[:], c1[:], p3[:], A_OP.subtract)
                        t1 = cmb.tile([128, CH], f32, name=f"t1{tn}{dt_}{ch}", tag="t1")
                        t2 = cmb.tile([128, CH], f32, name=f"t2{tn}{dt_}{ch}", tag="t2")
                        nc.vector.tensor_tensor(t1[:], y_r[:], cosT[:, csl], A_OP.mult)
                        nc.vector.tensor_tensor(t2[:], y_i[:], sinT[:, csl], A_OP.mult)
                        nc.vector.tensor_tensor(qrot[(tn, dt_, "r")][:, csl], t1[:], t2[:],
                                                A_OP.subtract)
                        nc.vector.tensor_tensor(t1[:], y_i[:], cosT[:, csl], A_OP.mult)
                        nc.vector.tensor_tensor(t2[:], y_r[:], sinT[:, csl], A_OP.mult)
                        nc.vector.tensor_tensor(qrot[(tn, dt_, "i")][:, csl], t1[:], t2[:],
                                                A_OP.add)

                # V: out[tok 128, d_out DS]; M1=xs*Vi, M2=xr*(Vr-Vi), M3=xi*(Vr+Vi)
                for tt in range(CH // 128):
                    bk = ch * (CH // 128) + tt
                    tsl = slice(tt * 128, tt * 128 + 128)
                    p1, p2, p3 = psum((128, DS)), psum((128, DS)), psum((128, DS))
                    for h in range(HT):
                        st = (h == 0); sp = (h == HT - 1)
                        nc.tensor.matmul(p1[:], acts["s"][:, h, tsl], wqk[("v", 1)][:, h, :],
                                         start=st, stop=sp)
                    for h in range(HT):
                        st = (h == 0); sp = (h == HT - 1)
                        nc.tensor.matmul(p2[:], acts["r"][:, h, tsl], wqk[("v", 2)][:, h, :],
                                         start=st, stop=sp)
                    for h in range(HT):
                        st = (h == 0); sp = (h == HT - 1)
                        nc.tensor.matmul(p3[:], acts["i"][:, h, tsl], wqk[("v", 3)][:, h, :],
                                         start=st, stop=sp)
                    cv = cmb.tile([128, DS], f32, name=f"cv{ch}{tt}", tag="cv")
                    nc.scalar.activation(cv[:], p1[:], mybir.ActivationFunctionType.Copy)
                    for hd in range(HPC):
                        hsl = slice(hd * 128, hd * 128 + 128)
                        nc.vector.tensor_tensor(vjoin[(hd, bk)][:, 0:128],
                                                cv[:, hsl], p2[:, hsl], A_OP.add)
                        nc.vector.tensor_tensor(vjoin[(hd, bk)][:, 128:256],
                                                cv[:, hsl], p3[:, hsl], A_OP.subtract)

        with tc.tile_pool(name="wo", bufs=2) as wo_pool:
            wo_pool_box.append(wo_pool)
            # preload first two o-proj weight blocks (run during attention)
            wo_pre = {0: load_wo_jb(0), 1: load_wo_jb(1)}

            # ======== phase 2: attention per head + per-head AllToAll ========
            SC = float(1.0 / np.sqrt(2 * D))
            with tc.tile_pool(name="attn", bufs=1) as at, \
                 tc.tile_pool(name="epool", bufs=2) as ep, \
                 tc.tile_pool(name="tp", bufs=2) as tp:
                for hd in range(HPC):
                    out_nat = {}

                    def emit_qk(g):
                        ets = {}
                        for bk in range(4 * g + 4):
                            pS = psum_at(bk % 4)
                            qsl = slice(g * 512, g * 512 + 512)
                            nc.tensor.matmul(pS[:],
                                             qrot[("k", hd, "r")][:, bk * 128:bk * 128 + 128],
                                             qrot[("q", hd, "r")][:, qsl],
                                             start=True, stop=False)
                            nc.tensor.matmul(pS[:],
                                             qrot[("k", hd, "i")][:, bk * 128:bk * 128 + 128],
                                             qrot[("q", hd, "i")][:, qsl],
                                             start=False, stop=True)
                            e = ep.tile([128, 512], bf16, name=f"e{hd}{g}_{bk}", tag=f"e{bk}")
                            nc.scalar.activation(e[:], pS[:],
                                                 mybir.ActivationFunctionType.Exp, scale=SC)
                            if bk >= 4 * g:
                                # diagonal block: cols < c0 invalid, triangle at c0
                                c0 = (bk - 4 * g) * 128
                                nc.vector.tensor_tensor(e[:, c0:c0 + 128], e[:, c0:c0 + 128],
                                                        tri[:], A_OP.mult)
                                if c0 > 0:
                                    nc.vector.memset(e[:, 0:c0], 0.0)
                            ets[bk] = e
                        return ets

                    def emit_po(g, ets):
                        for bq in range(4 * g, 4 * g + 4):
                            pO = psum_at(4 + bq % 4, (128, 257))
                            col = (bq - 4 * g) * 128
                            for bk in range(bq + 1):
                                nc.tensor.matmul(pO[:], ets[bk][:, col:col + 128],
                                                 vjoin[(hd, bk)][:],
                                                 start=(bk == 0), stop=(bk == bq))
                            rec = at.tile([128, 1], f32, name=f"rec{hd}{bq}")
                            nc.vector.reciprocal(rec[:], pO[:, 256:257])
                            for ci, cp in enumerate(("r", "i")):
                                o = at.tile([128, 128], f16, name=f"on{hd}{cp}{bq}",
                                            tag=f"on{cp}{bq}")
                                nc.vector.tensor_scalar(o[:], pO[:, ci * 128:ci * 128 + 128],
                                                        rec[:], None, A_OP.mult)
                                out_nat[(cp, bq)] = o
                                mx = at.tile([128, 1], f16, name=f"mx{hd}{cp}{bq}")
                                nc.vector.tensor_reduce(mx[:], o[:], mybir.AxisListType.X,
                                                        A_OP.max, apply_absolute_value=True)
                                s_, c0 = bq // 2, (bq % 2) * 128
                                r_mx = s_ * (HPC * R2) + hd * R2 + 2 * D + ci
                                dst = cont[r_mx: r_mx + 1, c0:c0 + 128]
                                nc.sync.dma_start(dst.rearrange("a b -> b a"), mx[:])

                    prev = None
                    for g in range(4):
                        ets = emit_qk(g)
                        if prev is not None:
                            emit_po(g - 1, prev)
                        prev = ets
                    emit_po(3, prev)

                    # transpose to [d, tok] and scatter into cont[hd]
                    for ci, cp in enumerate(("r", "i")):
                        oT = tp.tile([128, T], f16, name=f"oT{hd}{cp}", tag="oT")
                        for bq in range(T // 128):
                            pT = psum_at(bq % 2, (128, 128), f16)
                            nc.tensor.transpose(pT[:], out_nat[(cp, bq)][:], ident[:])
                            nc.vector.tensor_copy(oT[:, bq * 128:bq * 128 + 128], pT[:])
                        r0 = ci * 128
                        for s_ in range(NC):
                            rb = s_ * (HPC * R2) + hd * R2 + r0
                            nc.sync.dma_start(cont[rb: rb + 128, :],
                                              oT[:, s_ * TS:(s_ + 1) * TS])
                nc.gpsimd.collective_compute(
                    "AllToAll", A_OP.bypass, replica_groups=[list(range(NC))],
                    ins=[cont[:].opt()], outs=[ag[:].opt()])

            # ======== phase 3: o-projection on my 256-token slice ========
            with tc.tile_pool(name="op", bufs=1) as op, \
                 tc.tile_pool(name="od", bufs=2) as od:
                # global per-token absmax over 16 (core, head) sources
                gmax = {}
                for ci, cp in enumerate(("r", "i")):
                    g = op.tile([1, TS], f32, name=f"gmax{cp}")
                    first = True
                    agrows = ag[:].rearrange("(s r) c -> r s c", r=HPC * R2)
                    for hd in range(HPC):
                        rr = hd * R2 + 2 * D + ci
                        mrows = op.tile([1, NC, TS], f16, name=f"mrows{cp}{hd}", tag="mrows")
                        nc.sync.dma_start(mrows[:], agrows[rr: rr + 1, :, :])
                        for s_ in range(NC):
                            if first:
                                nc.vector.tensor_tensor(g[:], mrows[:, s_, :], mrows[:, s_, :],
                                                        A_OP.max)
                                first = False
                            else:
                                nc.vector.tensor_tensor(g[:], g[:], mrows[:, s_, :], A_OP.max)
                    nc.vector.tensor_scalar(g[:], g[:], 1e-5, None, A_OP.max)
                    gmax[cp] = g
                srep_s = {}; srep_inv = {}
                for ci, cp in enumerate(("r", "i")):
                    rg = op.tile([1, TS], f32, name=f"rg{cp}")
                    nc.vector.reciprocal(rg[:], gmax[cp][:])
                    nc.vector.tensor_scalar(rg[:], rg[:], 127.0, None, A_OP.mult)
                    nc.sync.dma_start(bounce[ci:ci + 1, :], rg[:])
                    iv = op.tile([1, TS], f32, name=f"iv{cp}")
                    nc.vector.tensor_scalar(iv[:], gmax[cp][:], float(1.0 / 127.0), None,
                                            A_OP.mult)
                    nc.sync.dma_start(bounce[2 + ci:3 + ci, :], iv[:])
                    sr = op.tile([128, TS], f32, name=f"sreps{cp}")
                    nc.sync.dma_start(sr[:], bounce[ci:ci + 1, :].to_broadcast((128, TS)))
                    srep_s[cp] = sr
                    si = op.tile([128, TS], f32, name=f"srepi{cp}")
                    nc.sync.dma_start(si[:], bounce[2 + ci:3 + ci, :].to_broadcast((128, TS)))
                    srep_inv[cp] = si

                # quantize+dequantize (and r+i sum) each 128-row contraction chunk
                # d16 = hfile*8 + c maps to ag[hfile] rows [c*R2 + cp*128 : +128]
                xq = {}
                for d16 in range(16):
                    hfile, c2 = d16 // 8, d16 % 8
                    for ci, cp in enumerate(("r", "i")):
                        r0 = c2 * (HPC * R2) + hfile * R2 + ci * 128
                        xt = op.tile([128, TS], f16, name=f"xt{cp}{d16}", tag="xt")
                        nc.sync.dma_start(xt[:], ag[r0:r0 + 128, :])
                        m = op.tile([128, TS], f32, name=f"xm{cp}{d16}", tag="xm")
                        nc.vector.tensor_tensor(m[:], xt[:], srep_s[cp][:], A_OP.mult)
                        r_ = op.tile([128, TS], f32, name=f"xr_{cp}{d16}", tag="xr_")
                        nc.vector.tensor_scalar(r_[:], m[:], MAGIC, MAGIC, A_OP.add,
                                                A_OP.subtract)
                        d_ = op.tile([128, TS], f16, name=f"xd{cp}{d16}")
                        nc.vector.tensor_tensor(d_[:], r_[:], srep_inv[cp][:], A_OP.mult)
                        xq[(cp, d16)] = d_
                    s_t = op.tile([128, TS], f16, name=f"xsum{d16}")
                    nc.vector.tensor_tensor(s_t[:], xq[("r", d16)][:], xq[("i", d16)][:],
                                            A_OP.add)
                    xq[("s", d16)] = s_t

                for jb in range(4):
                    w1, w2, w3 = wo_pre.pop(jb) if jb in wo_pre else load_wo_jb(jb)
                    pg = {}
                    for tt in range(2):
                        pg[(tt, 1)] = psum_at(3 * tt + 0)
                        pg[(tt, 2)] = psum_at(3 * tt + 1)
                        pg[(tt, 3)] = psum_at(3 * tt + 2)
                    for m, w, cp in ((1, w1, "s"), (2, w2, "r"), (3, w3, "i")):
                        for d16 in range(16):
                            st = (d16 == 0); sp = (d16 == 15)
                            for tt in range(2):
                                lhs = xq[(cp, d16)][:, tt * 128:(tt + 1) * 128]
                                nc.tensor.matmul(pg[(tt, m)][:], lhs, w[:, d16, :],
                                                 start=st, stop=sp)
                    for tt in range(2):
                        c1 = od.tile([128, 512], f32, name=f"oc1{jb}{tt}", tag="oc1")
                        nc.scalar.activation(c1[:], pg[(tt, 1)][:],
                                             mybir.ActivationFunctionType.Copy)
                        fr = od.tile([128, 512], f32, name=f"fr{jb}{tt}", tag="fr")
                        nc.vector.tensor_tensor(fr[:], c1[:], pg[(tt, 2)][:], A_OP.add)
                        nc.sync.dma_start(
                            A["yr_part"][tt * 128:(tt + 1) * 128, jb * 512:(jb + 1) * 512],
                            fr[:])
                        fi = od.tile([128, 512], f32, name=f"fi{jb}{tt}", tag="fi")
                        nc.vector.tensor_tensor(fi[:], c1[:], pg[(tt, 3)][:], A_OP.subtract)
                        nc.sync.dma_start(
                            A["yi_part"][tt * 128:(tt + 1) * 128, jb * 512:(jb + 1) * 512],
                            fi[:])


_CACHE = {}

def _get_compiled():
    if "nc" not in _CACHE:
        from contextlib import ExitStack
        nc, A = build_nc()
        with tile.TileContext(nc) as tc:
            with ExitStack() as ctx:
                emit(nc, A, tc, ctx)
        nc.compile()
        _CACHE["nc"] = nc
    return _CACHE["nc"]


def _prechunk(x, w):
    """[Hrows, Wcols] -> [Hrows/128 * 128, (Hrows/128-major) cols] contiguous blocks.

    out[blk*128 + p, h*w + j] = x[h*128 + p, blk*w + j]  -- wait, no: blocks over
    COLUMN windows of width w; h iterates row-chunks.
    """
    rows, cols = x.shape
    nb = cols // w
    hh = rows // 128
    out = np.empty((nb * 128, hh * w), dtype=x.dtype)
    xr = x.reshape(hh, 128, nb, w)
    for b in range(nb):
        out[b * 128:(b + 1) * 128, :] = (
            xr[:, :, b, :].transpose(1, 0, 2).reshape(128, hh * w))
    return out


def _host_prep(hidden_real, hidden_imag, positions,
               Wq_r, Wq_i, Wk_r, Wk_i, Wv_r, Wv_i, Wo_r, Wo_i):
    fp16 = np.float16
    f = np.float32
    hr = np.asarray(hidden_real, f); hi = np.asarray(hidden_imag, f)

    def quant(x):
        m = np.maximum(np.abs(x).max(1, keepdims=True), f(1e-5))
        s = (f(127.0) / m).astype(f)
        q = np.clip(np.rint(x * s), -128.0, 127.0).astype(f)
        return (q / s).astype(f)

    qr = quant(hr); qi = quant(hi); qs = qr + qi
    inv_freq = (f(1.0) / (f(10000.0) ** (np.arange(D, dtype=f) / f(D)))).astype(f)
    freqs = np.asarray(positions, np.int32).astype(f)[:, None] * inv_freq[None, :]
    cos = np.cos(freqs).astype(f).astype(ml_dtypes.bfloat16).astype(fp16)
    sin = np.sin(freqs).astype(f).astype(ml_dtypes.bfloat16).astype(fp16)
    tri = (np.arange(128)[None, :] >= np.arange(128)[:, None]).astype(fp16)

    # o-proj weight rows permuted to the device contraction order:
    # d16 = hfile*8 + c  ->  head (2c + hfile)
    perm = np.empty(H, np.int64)
    for d16 in range(16):
        hfile, c2 = d16 // 8, d16 % 8
        head = 2 * c2 + hfile
        perm[d16 * 128:(d16 + 1) * 128] = np.arange(head * 128, head * 128 + 128)
    Wor = np.asarray(Wo_r, f); Woi = np.asarray(Wo_i, f)
    asc = np.ascontiguousarray
    base = {
        "xr": _prechunk(qr.T.astype(fp16), CH),
        "xi": _prechunk(qi.T.astype(fp16), CH),
        "xs": _prechunk(qs.T.astype(fp16), CH),
        "wo1": _prechunk(Woi.T[perm].astype(fp16), 512),
        "wo2": _prechunk((Wor - Woi).T[perm].astype(fp16), 512),
        "wo3": _prechunk((Wor + Woi).T[perm].astype(fp16), 512),
        "cosT": asc(cos.T), "sinT": asc(sin.T),
        "tri": tri, "ident": np.eye(128, dtype=fp16),
    }
    in_maps = []
    for c in range(NC):
        sl = slice(c * DS, (c + 1) * DS)
        im = dict(base)
        for nm, Wr_, Wi_ in (("q", Wq_r, Wq_i), ("k", Wk_r, Wk_i), ("v", Wv_r, Wv_i)):
            Wr = np.asarray(Wr_, f)[sl]; Wi = np.asarray(Wi_, f)[sl]
            im[f"w{nm}1"] = _prechunk(Wi.T.astype(fp16), DS)
            im[f"w{nm}2"] = _prechunk((Wr - Wi).T.astype(fp16), DS)
            im[f"w{nm}3"] = _prechunk((Wr + Wi).T.astype(fp16), DS)
        in_maps.append(im)
    return in_maps


def kernel(**inputs):
    nc = _get_compiled()
    in_maps = _host_prep(**inputs)
    res = run_bass_kernel_spmd(nc, in_maps, list(range(NC)))
    yr = np.concatenate([res.results[c]["yr_part"] for c in range(NC)], axis=0)
    yi = np.concatenate([res.results[c]["yi_part"] for c in range(NC)], axis=0)
    return yr, yi
